# revision 1
# baseline (speedup 1.0000x reference)
"""BEiT-style attention (B=16, N=577, C=768, H=12) on 8 TRN2 NeuronCores.

Strategy: pure data-parallel over batch (2 batches/core, no collectives).
Per-core kernel computes attention in a transposed-score layout (S^T with
softmax axis on partitions) which needs zero on-device transposes:

  qT,kT  [d, n] = W_{q,k} @ x^T           (q pre-scaled, q_bias folded in)
  S^T    [m, n] = kT.T-slices @ qT        (K=64 contraction)
  expS^T [m, n] = exp(S^T) * exp(relposT) (exp(bias) precomputed on host)
  outT   [d, n] = [v | 1].T @ expS^T      (row 64 = softmax denominator)
  out    [n,co] = outT_norm.T-slices @ Wp^T + bias  (v_bias folded into bias)

All host-side prep (transposes, bf16 casts, exp of bias, bias folding) is
done in numpy inside kernel().
"""

import os
import sys
from contextlib import ExitStack

import numpy as np

sys.path.insert(0, "/opt/trn_rl_repo")

# the kernel executes through jax/PJRT on the axon-tunneled NeuronCores; a
# JAX_PLATFORMS=cpu pin (useful for pure-reference runs) would hide them
if os.environ.get("JAX_PLATFORMS", "") == "cpu":
    os.environ.pop("JAX_PLATFORMS", None)

import ml_dtypes  # noqa: E402

from concourse import bacc, mybir  # noqa: E402
import concourse.bass as bass  # noqa: E402
import concourse.tile as tile  # noqa: E402
from concourse.bass_utils import run_bass_kernel_spmd  # noqa: E402

BF16 = mybir.dt.bfloat16
F32 = mybir.dt.float32
NPBF16 = ml_dtypes.bfloat16
AF = mybir.ActivationFunctionType

B, N, C = 16, 577, 768
H, HD = 12, 64
NCORES = 8
BL = B // NCORES  # local batches per core
KC = C // 128  # contraction tiles over channels
SCALE = HD ** -0.5

# token-dim partition tiles (offset, width)
NT = [(0, 128), (128, 128), (256, 128), (384, 128), (512, 65)]
# token-dim free chunks (halves of 577, each fits one PSUM bank / <=512 mm)
FC = [(0, 289), (289, 288)]
# chunks aligned to PSUM bank boundary for fused two-bank [.,577] psum tiles
FB = [(0, 512), (512, 65)]
# channel free chunks for 768-wide outputs
CC = [(0, 384), (384, 384)]


def build_graph():
    nc = bacc.Bacc("TRN2", target_bir_lowering=False, debug=False, num_devices=NCORES)

    xT_d = nc.dram_tensor("xT", (BL, C, N), BF16, kind="ExternalInput").ap()
    wqk_d = nc.dram_tensor("wqkT", (C, 2 * C), BF16, kind="ExternalInput").ap()
    wv_d = nc.dram_tensor("wvT", (C, C), BF16, kind="ExternalInput").ap()
    pw_d = nc.dram_tensor("pwT", (C, C), BF16, kind="ExternalInput").ap()
    eb_d = nc.dram_tensor("ebT", (H, N, N), BF16, kind="ExternalInput").ap()
    qkb_d = nc.dram_tensor("qkb", (128, 2 * KC), F32, kind="ExternalInput").ap()
    pbc_d = nc.dram_tensor("pbc", (128, KC), F32, kind="ExternalInput").ap()
    out_d = nc.dram_tensor("out", (BL, C, N), F32, kind="ExternalOutput").ap()

    with tile.TileContext(nc) as tc, ExitStack() as ctx:
        res = ctx.enter_context(tc.tile_pool(name="res", bufs=1))
        ebp = ctx.enter_context(tc.tile_pool(name="ebp", bufs=4))
        e1p = ctx.enter_context(tc.tile_pool(name="e1p", bufs=6))
        estp = ctx.enter_context(tc.tile_pool(name="estp", bufs=8))
        rowp = ctx.enter_context(tc.tile_pool(name="rowp", bufs=6))
        bcp = ctx.enter_context(tc.tile_pool(name="bcp", bufs=6))
        finp = ctx.enter_context(tc.tile_pool(name="finp", bufs=4))
        ps_mm = ctx.enter_context(
            tc.tile_pool(name="ps_mm", bufs=2, space=bass.MemorySpace.PSUM)
        )
        ps_st = ctx.enter_context(
            tc.tile_pool(name="ps_st", bufs=2, space=bass.MemorySpace.PSUM)
        )
        ps_pv = ctx.enter_context(
            tc.tile_pool(name="ps_pv", bufs=2, space=bass.MemorySpace.PSUM)
        )

        # ---- resident tiles + batched input DMA (issued from idle SP) ----
        wqk = res.tile([128, KC * 2 * C], BF16, name="wqk", tag="wqk")
        wv = res.tile([128, KC * C], BF16, name="wv", tag="wv")
        pw = res.tile([128, KC * C], BF16, name="pw", tag="pw")
        xt = [res.tile([128, KC * N], BF16, name=f"xt{b}", tag=f"xt{b}") for b in range(BL)]
        qkt = [
            [res.tile([128, N], BF16, name=f"qkt{b}_{t}", tag=f"qkt{b}_{t}") for t in range(2 * KC)]
            for b in range(BL)
        ]
        vt = [
            [res.tile([128, H * (HD + 1)], BF16, name=f"vt{b}_{m}", tag=f"vt{b}_{m}") for m in range(len(NT))]
            for b in range(BL)
        ]
        ot = [
            [res.tile([128, N], BF16, name=f"ot{b}_{k}", tag=f"ot{b}_{k}") for k in range(KC)]
            for b in range(BL)
        ]
        qkb = res.tile([128, 2 * KC], F32, name="qkb_s", tag="qkb_s")
        pbc = res.tile([128, KC], F32, name="pbc_s", tag="pbc_s")

        def dma3(dst, src_2d, blk, k, eng=None):
            """DRAM rows (k*128 .. k*128+128, blk) -> SBUF tile cols [k*blk:(k+1)*blk]."""
            (eng or nc.sync).dma_start(
                dst[:, k * blk : (k + 1) * blk],
                src_2d[k * 128 : (k + 1) * 128, :],
            )

        # split issue across SP and the (startup-idle) GpSimd queue: the
        # ~1us per-dma issue cost serializes per engine, and wqk gates the
        # first qk matmuls
        for k in range(KC):
            dma3(xt[0], xT_d[0], N, k)
            dma3(wv, wv_d, C, k)
        for k in range(KC):
            dma3(wqk, wqk_d, 2 * C, k)
        nc.sync.dma_start(qkb[:], qkb_d[:])
        for k in range(KC):
            dma3(xt[1], xT_d[1], N, k)

        def xts(b, k):  # xT k-tile view [128, N]
            return xt[b][:, k * N : (k + 1) * N]

        # ones column (index HD) interleaved per head in the v tiles
        for b in range(BL):
            for m in range(len(NT)):
                vints = vt[b][m][:].rearrange("p (h e) -> p h e", h=H)
                nc.vector.memset(vints[:, :, HD : HD + 1], 1.0)

        def emit_qkv_v(b, m):
            m0, mw = NT[m]
            for ci, (c0, cw) in enumerate(CC):
                ps = ps_mm.tile([128, 512], F32, name=f"psv{b}_{m}_{ci}", tag="mm")
                for k in range(KC):
                    nc.tensor.matmul(
                        ps[:mw, :cw],
                        xts(b, k)[:, m0 : m0 + mw],
                        wv[:, k * C + c0 : k * C + c0 + cw],
                        start=(k == 0),
                        stop=(k == KC - 1),
                    )
                nh = cw // HD
                dst = vt[b][m][:mw, ci * nh * (HD + 1) : (ci + 1) * nh * (HD + 1)]
                dst = dst.rearrange("p (h e) -> p h e", h=nh)[:, :, 0:HD]
                src = ps[:mw, :cw].rearrange("p (h e) -> p h e", h=nh)
                nc.vector.tensor_copy(dst, src)

        def emit_qkv_qk(b, ct, ts=None):
            for t in ts if ts is not None else (ct, KC + ct):
                for n0, nw in FC:
                    ps = ps_mm.tile([128, 512], F32, name=f"psqk{b}_{t}_{n0}", tag="mm")
                    for k in range(KC):
                        nc.tensor.matmul(
                            ps[:, :nw],
                            wqk[:, k * 2 * C + t * 128 : k * 2 * C + (t + 1) * 128],
                            xts(b, k)[:, n0 : n0 + nw],
                            start=(k == 0),
                            stop=(k == KC - 1),
                        )
                    sc = SCALE if t < KC else 1.0
                    # DVE, not ACT: keeps the scalar engine exp-only so the
                    # activation LUT never swaps mid-kernel on hardware
                    nc.vector.tensor_scalar(
                        qkt[b][t][:, n0 : n0 + nw],
                        ps[:, :nw],
                        sc,
                        qkb[:, t : t + 1],
                        mybir.AluOpType.mult,
                        mybir.AluOpType.add,
                    )

        def emit_eb(h, tagsfx):
            eb = ebp.tile([128, 5 * N], BF16, name=f"eb{h}{tagsfx}", tag="eb")
            nc.sync.dma_start(
                eb[:, 0 : 4 * N].rearrange("p (m n) -> p m n", m=4),
                eb_d[h, 0:512, :].rearrange("(m p) n -> p m n", p=128),
            )
            nc.sync.dma_start(eb[0:65, 4 * N : 5 * N], eb_d[h, 512:577, :])
            return eb

        def emit_att_st(h, b, eb):
            """S^T matmuls + exp + bias-mult for one head; returns est pair tiles."""
            ctq = h // 2
            off = (h % 2) * HD
            est = []   # per-pair tiles [128, 2N]; est slice for mt m = pair[m//2][:, (m%2)*N:]
            pe1 = pcur = None
            for m, (m0, mw) in enumerate(NT):
                if m % 2 == 0:
                    w = 2 * N if m + 1 < len(NT) else N
                    pcur = estp.tile([128, w], BF16, name=f"est{h}_{b}_{m}", tag="est")
                    pe1 = e1p.tile([128, w], BF16, name=f"e1{h}_{b}_{m}", tag="e1")
                    est.append(pcur)
                co = (m % 2) * N
                ps = ps_st.tile([128, N], F32, name=f"pst{h}_{b}_{m}", tag="st")
                for n0, nw in FB:
                    nc.tensor.matmul(
                        ps[:mw, n0 : n0 + nw],
                        qkt[b][KC + ctq][off : off + HD, m0 : m0 + mw],
                        qkt[b][ctq][off : off + HD, n0 : n0 + nw],
                        start=True,
                        stop=True,
                    )
                nc.scalar.activation(pe1[:mw, co : co + N], ps[:mw, :], AF.Exp)
                if m % 2 == 1 or m == len(NT) - 1:
                    pw_ = co + N
                    nc.vector.tensor_mul(
                        pcur[:mw, 0:pw_],
                        pe1[:mw, 0:pw_],
                        eb[:mw, (m - pw_ // N + 1) * N : (m + 1) * N],
                    )
            return est

        def emit_att_pv(h, b, est):
            ctq = h // 2
            off = (h % 2) * HD
            for fi, (n0, nw) in enumerate(FB):
                pv = ps_pv.tile([HD + 1, 512], F32, name=f"pv{h}_{b}_{fi}", tag="pv")
                for m, (m0, mw) in enumerate(NT):
                    sl = (m % 2) * N + n0
                    nc.tensor.matmul(
                        pv[: HD + 1, :nw],
                        vt[b][m][:mw, h * (HD + 1) : (h + 1) * (HD + 1)],
                        est[m // 2][:mw, sl : sl + nw],
                        start=(m == 0),
                        stop=(m == len(NT) - 1),
                    )
                rr = rowp.tile([1, 512], BF16, name=f"rr{h}_{b}_{fi}", tag="rr")
                with nc.allow_low_precision("softmax denominator recip in bf16"):
                    nc.vector.reciprocal(rr[0:1, :nw], pv[HD : HD + 1, :nw])
                bc = bcp.tile([HD, 512], BF16, name=f"bc{h}_{b}_{fi}", tag="bc")
                nc.gpsimd.partition_broadcast(bc[:, :nw], rr[0:1, :nw], channels=HD)
                nc.vector.tensor_mul(
                    ot[b][ctq][off : off + HD, n0 : n0 + nw], pv[0:HD, :nw], bc[:, :nw]
                )

        def emit_att(h, b, eb):
            emit_att_pv(h, b, emit_att_st(h, b, eb))

        def emit_proj(b, cot):
            """fin^T[co, n] = pw-block.T-slices @ ot; bias is per-partition
            here; host transposes the (C, N) output back to (N, C)."""
            fin = finp.tile([128, N], F32, name=f"fin{b}_{cot}", tag="fin")
            for n0, nw in FB:
                ps = ps_mm.tile([128, 512], F32, name=f"psp{b}_{cot}_{n0}", tag="mm")
                for k in range(KC):
                    nc.tensor.matmul(
                        ps[:, :nw],
                        pw[:, k * C + cot * 128 : k * C + (cot + 1) * 128],
                        ot[b][k][:, n0 : n0 + nw],
                        start=(k == 0),
                        stop=(k == KC - 1),
                    )
                # b1 drains after the last Exp -> ACT idle there (one LUT
                # switch); alternate ACT/DVE so neither serializes the tail;
                # b0's eviction overlaps exps -> keep on DVE
                if b == 1 and cot % 2 == 0:
                    nc.scalar.activation(
                        fin[:, n0 : n0 + nw],
                        ps[:, :nw],
                        AF.Identity,
                        bias=pbc[:, cot : cot + 1],
                    )
                else:
                    nc.vector.tensor_scalar_add(
                        fin[:, n0 : n0 + nw], ps[:, :nw], pbc[:, cot : cot + 1]
                    )
                nc.sync.dma_start(
                    out_d[b, cot * 128 : (cot + 1) * 128, n0 : n0 + nw],
                    fin[:, n0 : n0 + nw],
                )

        # ---- emission schedule: fill PE during ACT/DVE-bound attention ----
        # merged pass 1: b0 QKV feeds b0 attention head-pairs immediately so
        # the exp/mult/norm stream starts ~15us earlier; b1 QKV interleaved
        for m in range(len(NT)):
            emit_qkv_v(0, m)
        for ct in range(KC):
            emit_qkv_qk(0, ct)
            if ct < len(NT):
                emit_qkv_v(1, ct)
            else:
                emit_qkv_qk(1, 0)
            for h in (2 * ct, 2 * ct + 1):
                eb = emit_eb(h, "a")
                emit_att(h, 0, eb)
            if ct == 2:
                # proj weights aren't consumed until pass 2; issue them here
                # so they never queue ahead of the eb bias tiles on SP
                for k in range(KC):
                    dma3(pw, pw_d, C, k)
                nc.sync.dma_start(pbc[:], pbc_d[:])
        # pass 2: b1 attention; fillers: remaining b1 qk pairs (ct before head
        # 2ct) early, b0 proj (DVE-heavy epilogue) late
        for h in range(H):
            eb = emit_eb(h, "b")
            if h < 10:
                ct = h // 2 + 1
                emit_qkv_qk(1, ct, ts=(ct,) if h % 2 == 0 else (KC + ct,))
            if h >= 7:
                emit_proj(0, h - 7)
            emit_att(h, 1, eb)
        emit_proj(0, 5)
        for cot in range(KC):
            emit_proj(1, cot)

    nc.compile()
    return nc


_NC = None


def get_compiled():
    global _NC
    if _NC is None:
        _NC = build_graph()
    return _NC


def prep_in_maps(x, rel_pos_bias, qkv_weight, q_bias, v_bias, proj_weight, proj_bias):
    x = np.asarray(x, np.float32)
    rel_pos_bias = np.asarray(rel_pos_bias, np.float32)
    qkv_weight = np.asarray(qkv_weight, np.float32)
    q_bias = np.asarray(q_bias, np.float32)
    v_bias = np.asarray(v_bias, np.float32)
    proj_weight = np.asarray(proj_weight, np.float32)
    proj_bias = np.asarray(proj_bias, np.float32)
    xT = np.ascontiguousarray(x.transpose(0, 2, 1)).astype(NPBF16)  # (B, C, N)
    wqkT = np.ascontiguousarray(qkv_weight[: 2 * C].T).astype(NPBF16)  # (C, 2C)
    wvT = np.ascontiguousarray(qkv_weight[2 * C :].T).astype(NPBF16)  # (C, C)
    pwT = np.ascontiguousarray(proj_weight.T).astype(NPBF16)  # (C, C)
    ebT = np.exp(rel_pos_bias.transpose(0, 2, 1).astype(np.float64)).astype(NPBF16)

    qkb = np.zeros((128, 2 * KC), np.float32)
    for t in range(KC):
        qkb[:, t] = SCALE * q_bias[t * 128 : (t + 1) * 128]

    pbe = (proj_bias + v_bias @ proj_weight.T).astype(np.float32)  # (C,)
    pbc = np.ascontiguousarray(pbe.reshape(KC, 128).T)  # [p, cot] = pbe[cot*128+p]

    shared = {"wqkT": wqkT, "wvT": wvT, "pwT": pwT, "ebT": ebT, "qkb": qkb, "pbc": pbc}
    in_maps = []
    for i in range(NCORES):
        m = dict(shared)
        m["xT"] = np.ascontiguousarray(xT[i * BL : (i + 1) * BL])
        in_maps.append(m)
    return in_maps


def run(inputs, trace=False, **kw):
    nc = get_compiled()
    in_maps = prep_in_maps(**inputs)
    res = run_bass_kernel_spmd(nc, in_maps, core_ids=list(range(NCORES)), trace=trace, **kw)
    outT = np.concatenate([r["out"] for r in res.results], axis=0)  # (B, C, N)
    out = np.ascontiguousarray(outT.transpose(0, 2, 1))
    return out, res


def kernel(**inputs):
    out, _ = run(inputs, trace=False)
    return out

